# revision 22
# baseline (speedup 1.0000x reference)
"""GCN (2-layer Kipf-Welling) forward on 8 Trainium2 NeuronCores.

Strategy (graph/data parallel, destination-sharded per the hint):

  Launch A (transform): per-core node shard computes h' = dinv * (x @ W1)
    on TensorE (W1 moving, x^T tiles stationary), dinv = 1/sqrt(deg) on
    ScalarE+VectorE. Emits the bf16 h' table shard and dinv shard.

  Host: "halo exchange" is materialized by gathering h' rows into per-core,
    destination-bucketed edge streams (pure data movement; all arithmetic
    stays on device). Each stream row packs 4 same-destination edges
    (4 x 64 feats = 256 cols).

  Launch B (layer-1 aggregation + layer-2 transform): each block of 128
    destinations is two 64-dest windows; a one-hot selection matrix
    S [128 rows x 64 dests] is built on VectorE from per-row relative-dest
    ids and used as the stationary matmul operand, one wide N=256 matmul
    per tile accumulating PSUM[dest, 4x64] (window w lands on PSUM
    partitions [64w, 64w+64) via tile_position). Epilogue folds the 4
    edge-groups and computes h1 = relu(dinv*agg + b1), then
    z' = dinv * (h1 @ W2) via a fused multiply+row-sum. Only
    z' [128, NBLK] leaves the device.

  Host: gathers z' scalars into 32-packed per-core streams (movement only).

  Launch C (layer-2 aggregation): same one-hot matmul trick in f32,
    32 same-dest scalars per row; out = dinv * agg + b2.

All reference FLOPs (transform, normalization, aggregation, bias, relu)
run on device; the host only computes the schedule (integer index maps)
and moves bytes.
"""

import math

import numpy as np
import ml_dtypes

import concourse.bacc as bacc
import concourse.mybir as mybir
import concourse.tile as tile
from concourse._compat import get_trn_type

P = 128
N_CORES = 8
IN_DIM = 128
HID = 64
PACK_B = 4     # edges per stream row, layer-1 aggregation
PACK_C = 32    # edges per stream row, layer-2 aggregation
f32 = mybir.dt.float32
bf16 = mybir.dt.bfloat16

_COMPILE_CACHE = {}
_RUNNER_CACHE = {}


# ---------------------------------------------------------------------------
# minimal PJRT runner (self-contained; mirrors bass2jax.run_bass_via_pjrt but
# keeps the jitted callable + staged arrays reusable across calls)
# ---------------------------------------------------------------------------

class _Runner:
    def __init__(self, nc, n_cores=N_CORES):
        import jax
        from jax.sharding import Mesh, NamedSharding, PartitionSpec
        from jax.experimental.shard_map import shard_map
        from concourse.bass2jax import (
            _bass_exec_p,
            install_neuronx_cc_hook,
            partition_id_tensor,
        )

        install_neuronx_cc_hook()
        self.jax = jax
        self.nc = nc
        self.n_cores = n_cores
        partition_name = (
            nc.partition_id_tensor.name if nc.partition_id_tensor else None
        )
        in_names, out_names, out_avals, zero_outs = [], [], [], []
        for alloc in nc.m.functions[0].allocations:
            if not isinstance(alloc, mybir.MemoryLocationSet):
                continue
            name = alloc.memorylocations[0].name
            if alloc.kind == "ExternalInput":
                if name != partition_name:
                    in_names.append(name)
            elif alloc.kind == "ExternalOutput":
                shape = tuple(alloc.tensor_shape)
                dtype = mybir.dt.np(alloc.dtype)
                out_names.append(name)
                out_avals.append(jax.core.ShapedArray(shape, dtype))
                zero_outs.append(np.zeros(shape, dtype))
        self.in_names = in_names
        self.out_names = out_names
        self.out_shapes = [tuple(a.shape) for a in out_avals]
        self.zero_outs = zero_outs
        all_in_names = in_names + out_names
        if partition_name is not None:
            all_in_names.append(partition_name)

        def _body(*args):
            operands = list(args)
            if partition_name is not None:
                operands.append(partition_id_tensor())
            outs = _bass_exec_p.bind(
                *operands,
                out_avals=tuple(out_avals),
                in_names=tuple(all_in_names),
                out_names=tuple(out_names),
                lowering_input_output_aliases=(),
                sim_require_finite=True,
                sim_require_nnan=True,
                nc=nc,
            )
            return tuple(outs)

        devices = jax.devices()[:n_cores]
        assert len(devices) == n_cores
        self.mesh = Mesh(np.asarray(devices), ("core",))
        self.sharding = NamedSharding(self.mesh, PartitionSpec("core"))
        n_in = len(in_names) + len(out_names)
        self.fn = jax.jit(
            shard_map(
                _body,
                mesh=self.mesh,
                in_specs=(PartitionSpec("core"),) * n_in,
                out_specs=(PartitionSpec("core"),) * len(out_names),
                check_rep=False,
            ),
            keep_unused=True,
        )

    def stage(self, in_maps):
        """device_put per-core inputs (sharded along axis 0)."""
        jax = self.jax
        args = []
        for name in self.in_names:
            cat = np.concatenate(
                [np.ascontiguousarray(in_maps[c][name]) for c in range(self.n_cores)],
                axis=0,
            )
            args.append(jax.device_put(cat, self.sharding))
        for z in self.zero_outs:
            cat = np.zeros((self.n_cores * z.shape[0], *z.shape[1:]), z.dtype)
            args.append(jax.device_put(cat, self.sharding))
        jax.block_until_ready(args)
        return args

    def run(self, args):
        outs = self.fn(*args)
        self.jax.block_until_ready(outs)
        res = []
        for c in range(self.n_cores):
            d = {}
            for i, name in enumerate(self.out_names):
                full = np.asarray(outs[i])
                d[name] = full.reshape(
                    self.n_cores, full.shape[0] // self.n_cores, *full.shape[1:]
                )[c]
            res.append(d)
        return res


# ---------------------------------------------------------------------------
# host-side schedule (integer index maps only)
# ---------------------------------------------------------------------------

def _preprocess(edge_index, n_nodes):
    ei = np.asarray(edge_index)
    dst = ei[0].astype(np.int64)
    src = ei[1].astype(np.int64)
    loops = np.arange(n_nodes, dtype=np.int64)
    dst = np.concatenate([dst, loops])
    src = np.concatenate([src, loops])
    EE = dst.shape[0]

    deg = np.bincount(dst, minlength=n_nodes).astype(np.int64)  # >= 1

    NBLK = math.ceil(n_nodes / (N_CORES * P))
    n_cells = N_CORES * NBLK
    cap = n_cells * P

    # --- balanced dest -> (core, blk, part) assignment: snake by degree ---
    order = np.argsort(-deg, kind="stable")
    padded = np.concatenate([order, np.full(cap - n_nodes, -1, np.int64)])
    arr = padded.reshape(P, n_cells).copy()
    arr[1::2] = arr[1::2, ::-1]
    part_of = np.empty(n_nodes, np.int64)
    cell_of = np.empty(n_nodes, np.int64)
    rr, cc = np.nonzero(arr >= 0)
    part_of[arr[rr, cc]] = rr
    cell_of[arr[rr, cc]] = cc
    core_of = cell_of // NBLK
    blk0_of = cell_of % NBLK

    nB = -(-deg // PACK_B)
    nC = -(-deg // PACK_C)

    rowsB_cell = np.bincount(cell_of, weights=nB, minlength=n_cells)
    rowsB_2d = rowsB_cell.reshape(N_CORES, NBLK)
    order_b = np.argsort(-rowsB_2d, axis=1, kind="stable")
    newblk_map = np.empty((N_CORES, NBLK), np.int64)
    for c in range(N_CORES):
        newblk_map[c, order_b[c]] = np.arange(NBLK)
    blk_of = newblk_map[core_of, blk0_of]
    newcell = core_of * NBLK + blk_of

    # B uses 64-dest windows within each 128-dest block (halves the one-hot
    # build on VectorE); window = part//64, S cols = part%64.
    win_of = part_of // (P // 2)
    rowsBW = np.bincount(
        newcell * 2 + win_of, weights=nB, minlength=n_cells * 2
    ).reshape(N_CORES, NBLK, 2)
    rowsC_rel = np.bincount(newcell, weights=nC, minlength=n_cells).reshape(
        N_CORES, NBLK
    )
    twinB = np.maximum(
        1, -(-rowsBW.max(axis=0).astype(np.int64) // P)
    )  # [NBLK, 2]
    tbB = twinB.sum(axis=1)
    tbC = np.maximum(1, -(-rowsC_rel.max(axis=0).astype(np.int64) // P))
    prefB = np.concatenate([[0], np.cumsum(tbB)]).astype(np.int64)
    prefC = np.concatenate([[0], np.cumsum(tbC)]).astype(np.int64)
    TTB, TTC = int(prefB[-1]), int(prefC[-1])

    # per-dest row base within a group (dests ordered by part id)
    def rowbase(n_rows, group, n_groups):
        dorder = np.argsort(group * P + part_of, kind="stable")
        nr = n_rows[dorder]
        cum = np.cumsum(nr) - nr
        cells = group[dorder]
        first = np.zeros(n_groups, np.int64)
        firsts = np.concatenate([[True], cells[1:] != cells[:-1]])
        first[cells[firsts]] = cum[firsts]
        base = np.empty(n_nodes, np.int64)
        base[dorder] = cum - first[cells]
        return base

    baseB = rowbase(nB, newcell * 2 + win_of, n_cells * 2)
    baseC = rowbase(nC, newcell, n_cells)

    # edges grouped by dest
    eorder = np.argsort(dst, kind="stable")
    ds = dst[eorder]
    ss = src[eorder]
    starts = np.concatenate([[0], np.cumsum(deg)])
    within = np.arange(EE, dtype=np.int64) - starts[ds]

    pad_src = n_nodes  # zero row of the padded tables

    def build(pack, base, pref, TT, tile_off, rel_of):
        row = base[ds] + within // pack
        slot = within % pack
        t_glob = pref[blk_of[ds]] + tile_off[ds] + row // P
        p = row % P
        src_map = np.full((N_CORES, P, TT, pack), pad_src, np.int32)
        rel_map = np.full((N_CORES, P, TT), 200.0, np.float32)
        cd = core_of[ds]
        src_map[cd, p, t_glob, slot] = ss.astype(np.int32)
        rel_map[cd, p, t_glob] = rel_of[ds].astype(np.float32)
        return src_map, rel_map

    # window-1 rows start after window-0's tiles within the block
    w0_tiles = twinB[:, 0][blk_of] * win_of
    srcB, relB = build(
        PACK_B, baseB, prefB, TTB, w0_tiles, part_of % (P // 2)
    )
    zeros_off = np.zeros(n_nodes, np.int64)
    srcC, relC = build(PACK_C, baseC, prefC, TTC, zeros_off, part_of)

    dest_id = np.full((N_CORES, P, NBLK), -1, np.int64)
    dest_id[core_of, part_of, blk_of] = np.arange(n_nodes)

    # launch-A node layout: node = c*NBLK*P + t*P + p
    degA = np.ones((N_CORES, P, NBLK), np.float32)
    nodes = np.arange(n_nodes)
    degA[nodes // (NBLK * P), nodes % P, (nodes // P) % NBLK] = deg

    return dict(
        NBLK=NBLK, tbB=tbB, tbC=tbC, twinB=twinB, prefB=prefB, prefC=prefC,
        TTB=TTB, TTC=TTC, srcB=srcB, relB=relB, srcC=srcC, relC=relC,
        dest_id=dest_id, degA=degA,
    )


# ---------------------------------------------------------------------------
# device programs
# ---------------------------------------------------------------------------

def _build_A(NBLK, reps=1):
    """h' = dinv * (x @ W1) in bf16; also emits dinv."""
    nc = bacc.Bacc(get_trn_type() or "TRN2", debug=False)
    xT = nc.dram_tensor("xT", [P, NBLK * P], bf16, kind="ExternalInput")
    degA = nc.dram_tensor("degA", [P, NBLK], f32, kind="ExternalInput")
    w1 = nc.dram_tensor("w1", [IN_DIM, HID], bf16, kind="ExternalInput")
    hA = nc.dram_tensor("hA", [P, NBLK, HID], bf16, kind="ExternalOutput")
    dinvA = nc.dram_tensor("dinvA", [P, NBLK], f32, kind="ExternalOutput")

    CH = 14  # tiles per x-chunk (98 = 7 x 14)
    with tile.TileContext(nc) as tc:
        with (
            tc.tile_pool(name="const", bufs=1) as constp,
            tc.tile_pool(name="xc", bufs=3) as xcp,
            tc.tile_pool(name="stage", bufs=1) as stagep,
            tc.tile_pool(name="ps", bufs=4, space="PSUM") as psp,
        ):
            degb = constp.tile([P, NBLK], f32)
            w1b = constp.tile([IN_DIM, HID], bf16)
            sq = constp.tile([P, NBLK], f32)
            dinvb = constp.tile([P, NBLK], f32)
            stageb = stagep.tile([P, NBLK, HID], bf16)
            nc.sync.dma_start(degb[:], degA[:])
            nc.sync.dma_start(w1b[:], w1[:])
            for _ in range(reps):
                nc.scalar.activation(
                    out=sq[:], in_=degb[:],
                    func=mybir.ActivationFunctionType.Sqrt,
                )
                nc.vector.reciprocal(out=dinvb[:], in_=sq[:])
                for t0 in range(0, NBLK, CH):
                    nch = min(CH, NBLK - t0)
                    xc = xcp.tile([P, nch * P], bf16, tag="xc")
                    nc.sync.dma_start(
                        xc[:], xT[:, t0 * P : (t0 + nch) * P]
                    )
                    for tt in range(nch):
                        t = t0 + tt
                        ps = psp.tile([P, HID], f32, tag="ps")
                        nc.tensor.matmul(
                            ps[:], xc[:, tt * P : (tt + 1) * P], w1b[:],
                            start=True, stop=True,
                        )
                        nc.vector.tensor_scalar(
                            out=stageb[:, t, :], in0=ps[:],
                            scalar1=dinvb[:, t : t + 1], scalar2=None,
                            op0=mybir.AluOpType.mult,
                        )
            nc.sync.dma_start(hA[:], stageb[:])
            nc.sync.dma_start(dinvA[:], dinvb[:])
    nc.compile()
    return nc


def _build_B(twinB, prefB, reps=1):
    """Layer-1 aggregation (4-packed bf16 messages) + epilogue -> z'.

    Each 128-dest block is two 64-dest windows; the one-hot S has 64
    columns and window w's matmuls land on PSUM partitions [64w, 64w+64)
    via tile_position.
    """
    NBLK = len(twinB)
    tbB = [int(t0 + t1) for t0, t1 in twinB]
    TTB = int(prefB[-1])
    HW2 = P // 2
    GW = PACK_B * HID  # stream row width (256)
    nc = bacc.Bacc(get_trn_type() or "TRN2", debug=False)
    msgs = nc.dram_tensor("msgs", [P, TTB, GW], bf16, kind="ExternalInput")
    rel = nc.dram_tensor("rel", [P, TTB], bf16, kind="ExternalInput")
    iota = nc.dram_tensor("iota", [P, P], bf16, kind="ExternalInput")
    w2r = nc.dram_tensor("w2r", [P, HID], f32, kind="ExternalInput")
    b1r = nc.dram_tensor("b1r", [P, HID], f32, kind="ExternalInput")
    dinvd = nc.dram_tensor("dinvd", [P, NBLK], f32, kind="ExternalInput")
    zp = nc.dram_tensor("zp", [P, NBLK], f32, kind="ExternalOutput")

    pairs = [(b, min(b + 8, NBLK)) for b in range(0, NBLK, 8)]

    with tile.TileContext(nc) as tc:
        with (
            tc.tile_pool(name="const", bufs=1) as constp,
            tc.tile_pool(name="msg", bufs=3) as msgp,
            tc.tile_pool(name="sb", bufs=3) as sp,
            tc.tile_pool(name="f1", bufs=3) as f1p,
            tc.tile_pool(name="t1", bufs=6) as t1p,
            tc.tile_pool(name="h1", bufs=6) as h1p,
            tc.tile_pool(name="scr", bufs=6) as scrp,
            tc.tile_pool(name="ps", bufs=8, space="PSUM") as psp,
        ):
            relb = constp.tile([P, TTB], bf16)
            iotab = constp.tile([P, P], bf16)
            w2b = constp.tile([P, HID], f32)
            b1b = constp.tile([P, HID], f32)
            dinvb = constp.tile([P, NBLK], f32)
            zacc = constp.tile([P, NBLK], f32)
            zstage = constp.tile([P, NBLK], f32)
            nc.sync.dma_start(relb[:], rel[:])
            nc.sync.dma_start(iotab[:], iota[:])
            nc.sync.dma_start(w2b[:], w2r[:])
            nc.sync.dma_start(b1b[:], b1r[:])
            nc.sync.dma_start(dinvb[:], dinvd[:])

            for _ in range(reps):
                for b0, b1e in pairs:
                    off = int(prefB[b0])
                    tbsum = int(prefB[b1e] - prefB[b0])
                    mb = msgp.tile([P, tbsum, GW], bf16, tag="msg")
                    nc.sync.dma_start(mb[:], msgs[:, off : off + tbsum, :])
                    sb = sp.tile([P, tbsum, HW2], bf16, tag="s")
                    nc.vector.tensor_tensor(
                        out=sb[:],
                        in0=relb[:, off : off + tbsum, None].to_broadcast(
                            [P, tbsum, HW2]
                        ),
                        in1=iotab[:, None, 0:HW2].to_broadcast(
                            [P, tbsum, HW2]
                        ),
                        op=mybir.AluOpType.is_equal,
                    )
                    j0 = 0
                    for b in range(b0, b1e):
                        t0, t1 = int(twinB[b][0]), int(twinB[b][1])
                        tb = t0 + t1
                        pb = psp.tile([P, GW], f32, tag="pb")
                        for j in range(tb):
                            w = 0 if j < t0 else 1
                            nc.tensor.matmul(
                                pb[w * HW2 : (w + 1) * HW2, :],
                                sb[:, j0 + j, :],
                                mb[:, j0 + j, :],
                                start=(j == 0 or j == t0),
                                stop=(j == t0 - 1 or j == tb - 1),
                                tile_position=(0, w * HW2),
                            )
                        j0 += tb
                        # fold 4 edge-groups + scale + bias:
                        # t1 = dinv*(sum_g pb[:, g*64:...]) + b1, one PSUM
                        # read per op (DVE has a single PSUM read port)
                        t1 = t1p.tile([P, HID], f32, tag="t1")
                        nc.vector.scalar_tensor_tensor(
                            out=t1[:], in0=pb[:, 0:HID],
                            scalar=dinvb[:, b : b + 1], in1=b1b[:],
                            op0=mybir.AluOpType.mult,
                            op1=mybir.AluOpType.add,
                        )
                        for g in range(1, PACK_B):
                            nc.vector.scalar_tensor_tensor(
                                out=t1[:], in0=pb[:, g * HID : (g + 1) * HID],
                                scalar=dinvb[:, b : b + 1], in1=t1[:],
                                op0=mybir.AluOpType.mult,
                                op1=mybir.AluOpType.add,
                            )
                        h1 = h1p.tile([P, HID], f32, tag="h1")
                        nc.scalar.activation(
                            out=h1[:], in_=t1[:],
                            func=mybir.ActivationFunctionType.Relu,
                        )
                        scr = scrp.tile([P, HID], f32, tag="scr")
                        nc.vector.scalar_tensor_tensor(
                            out=scr[:], in0=h1[:], scalar=1.0, in1=w2b[:],
                            op0=mybir.AluOpType.mult,
                            op1=mybir.AluOpType.mult,
                            accum_out=zacc[:, b : b + 1],
                        )
                        nc.vector.tensor_scalar(
                            out=zstage[:, b : b + 1],
                            in0=zacc[:, b : b + 1],
                            scalar1=dinvb[:, b : b + 1], scalar2=None,
                            op0=mybir.AluOpType.mult,
                        )
            nc.sync.dma_start(zp[:], zstage[:])
    nc.compile()
    return nc


def _build_C(tbC, prefC, reps=1):
    """Layer-2 scalar aggregation (32-packed f32) -> out = dinv*agg + b2."""
    NBLK = len(tbC)
    TTC = int(prefC[-1])
    nc = bacc.Bacc(get_trn_type() or "TRN2", debug=False)
    msgs = nc.dram_tensor("msgs", [P, TTC, PACK_C], bf16, kind="ExternalInput")
    rel = nc.dram_tensor("rel", [P, TTC], bf16, kind="ExternalInput")
    iota = nc.dram_tensor("iota", [P, P], bf16, kind="ExternalInput")
    dinvd = nc.dram_tensor("dinvd", [P, NBLK], f32, kind="ExternalInput")
    b2r = nc.dram_tensor("b2r", [P, 1], f32, kind="ExternalInput")
    outv = nc.dram_tensor("outv", [P, NBLK], f32, kind="ExternalOutput")

    pairs = [(b, min(b + 8, NBLK)) for b in range(0, NBLK, 8)]

    with tile.TileContext(nc) as tc:
        with (
            tc.tile_pool(name="const", bufs=1) as constp,
            tc.tile_pool(name="msg", bufs=3) as msgp,
            tc.tile_pool(name="sb", bufs=3) as sp,
            tc.tile_pool(name="acc", bufs=3) as accp,
            tc.tile_pool(name="ps", bufs=4, space="PSUM") as psp,
        ):
            relb = constp.tile([P, TTC], bf16)
            iotab = constp.tile([P, P], bf16)
            dinvb = constp.tile([P, NBLK], f32)
            b2b = constp.tile([P, 1], f32)
            ostage = constp.tile([P, NBLK], f32)
            nc.sync.dma_start(relb[:], rel[:])
            nc.sync.dma_start(iotab[:], iota[:])
            nc.sync.dma_start(dinvb[:], dinvd[:])
            nc.sync.dma_start(b2b[:], b2r[:])

            for _ in range(reps):
                for b0, b1e in pairs:
                    off = int(prefC[b0])
                    tbsum = int(prefC[b1e] - prefC[b0])
                    mb = msgp.tile([P, tbsum, PACK_C], bf16, tag="msg")
                    nc.sync.dma_start(mb[:], msgs[:, off : off + tbsum, :])
                    sb = sp.tile([P, tbsum, P], bf16, tag="s")
                    nc.vector.tensor_tensor(
                        out=sb[:],
                        in0=relb[:, off : off + tbsum, None].to_broadcast(
                            [P, tbsum, P]
                        ),
                        in1=iotab[:, None, :].to_broadcast([P, tbsum, P]),
                        op=mybir.AluOpType.is_equal,
                    )
                    j0 = 0
                    for b in range(b0, b1e):
                        tb = int(tbC[b])
                        pc = psp.tile([P, PACK_C], f32, tag="pc")
                        for j in range(tb):
                            nc.tensor.matmul(
                                pc[:], sb[:, j0 + j, :], mb[:, j0 + j, :],
                                start=(j == 0), stop=(j == tb - 1),
                            )
                        j0 += tb
                        acc = accp.tile([P, 1], f32, tag="acc")
                        nc.vector.tensor_reduce(
                            out=acc[:], in_=pc[:],
                            axis=mybir.AxisListType.X,
                            op=mybir.AluOpType.add,
                        )
                        nc.vector.tensor_scalar(
                            out=ostage[:, b : b + 1], in0=acc[:],
                            scalar1=dinvb[:, b : b + 1], scalar2=b2b[:],
                            op0=mybir.AluOpType.mult,
                            op1=mybir.AluOpType.add,
                        )
            nc.sync.dma_start(outv[:], ostage[:])
    nc.compile()
    return nc


# ---------------------------------------------------------------------------
# kernel entry
# ---------------------------------------------------------------------------

def _get_programs(pp, reps=(1, 1, 1)):
    key = (pp["NBLK"], tuple(map(tuple, pp["twinB"])), tuple(pp["tbC"]), tuple(reps))
    if key not in _COMPILE_CACHE:
        import time as _t

        t0 = _t.time()
        ncA = _build_A(pp["NBLK"], reps=reps[0])
        t1 = _t.time()
        ncB = _build_B(pp["twinB"], pp["prefB"], reps=reps[1])
        t2 = _t.time()
        ncC = _build_C(pp["tbC"], pp["prefC"], reps=reps[2])
        t3 = _t.time()
        print(
            f"[kernel] compiled A {t1-t0:.0f}s B {t2-t1:.0f}s C {t3-t2:.0f}s",
            flush=True,
        )
        _COMPILE_CACHE[key] = (ncA, ncB, ncC)
    return _COMPILE_CACHE[key]


def _get_runner(nc, tag):
    if tag not in _RUNNER_CACHE or _RUNNER_CACHE[tag].nc is not nc:
        _RUNNER_CACHE[tag] = _Runner(nc)
    return _RUNNER_CACHE[tag]


def _prep_inputs_A(x, W1, pp):
    n_nodes = x.shape[0]
    NBLK = pp["NBLK"]
    capc = NBLK * P
    x_pad = np.zeros((N_CORES * capc, IN_DIM), np.float32)
    x_pad[:n_nodes] = np.asarray(x, np.float32)
    w1b = np.asarray(W1, np.float32).astype(ml_dtypes.bfloat16)
    maps = []
    for c in range(N_CORES):
        xT = np.ascontiguousarray(
            x_pad[c * capc : (c + 1) * capc].T
        ).astype(ml_dtypes.bfloat16)
        maps.append({"xT": xT, "degA": pp["degA"][c], "w1": w1b})
    return maps


def _prep_inputs_B(h_pad, W2, b1, dinv_d, pp):
    iota = np.tile(np.arange(P, dtype=np.float32), (P, 1)).astype(
        ml_dtypes.bfloat16
    )
    w2r = np.tile(np.asarray(W2, np.float32)[:, 0][None, :], (P, 1))
    b1r = np.tile(np.asarray(b1, np.float32)[None, :], (P, 1))
    relB = pp["relB"].astype(ml_dtypes.bfloat16)
    maps = []
    for c in range(N_CORES):
        msgs = h_pad[pp["srcB"][c]]  # [P, TTB, 4, 64] bf16
        maps.append(
            {
                "msgs": msgs.reshape(P, pp["TTB"], PACK_B * HID),
                "rel": relB[c],
                "iota": iota,
                "w2r": w2r,
                "b1r": b1r,
                "dinvd": dinv_d[c],
            }
        )
    return maps


def _prep_inputs_C(z_pad, b2, dinv_d, pp):
    iota = np.tile(np.arange(P, dtype=np.float32), (P, 1)).astype(
        ml_dtypes.bfloat16
    )
    b2r = np.full((P, 1), float(np.asarray(b2).reshape(-1)[0]), np.float32)
    z_bf = z_pad.astype(ml_dtypes.bfloat16)
    relC = pp["relC"].astype(ml_dtypes.bfloat16)
    maps = []
    for c in range(N_CORES):
        msgs = z_bf[pp["srcC"][c]]  # [P, TTC, 32] bf16
        maps.append(
            {
                "msgs": msgs,
                "rel": relC[c],
                "iota": iota,
                "dinvd": dinv_d[c],
                "b2r": b2r,
            }
        )
    return maps


_LAST = {}


def kernel(x, W1, b1, W2, b2, edge_index):
    x = np.asarray(x)
    n_nodes = x.shape[0]
    pp = _preprocess(edge_index, n_nodes)
    NBLK = pp["NBLK"]
    capc = NBLK * P

    ncA, ncB, ncC = _get_programs(pp)

    # ---- launch A: h' = dinv * (x @ W1)
    rA = _get_runner(ncA, "A")
    argsA = rA.stage(_prep_inputs_A(x, W1, pp))
    resA = rA.run(argsA)
    hA = np.stack([r["hA"] for r in resA])        # [8, P, NBLK, 64] bf16
    dinvA = np.stack([r["dinvA"] for r in resA])  # [8, P, NBLK] f32
    # node = c*capc + t*P + p
    h_pad = np.ascontiguousarray(
        hA.transpose(0, 2, 1, 3).reshape(N_CORES * capc, HID)
    )
    dinv_full = dinvA.transpose(0, 2, 1).reshape(N_CORES * capc)
    h_pad[n_nodes:] = 0
    dvalid = pp["dest_id"] >= 0
    dinv_d = np.where(
        dvalid, dinv_full[np.where(dvalid, pp["dest_id"], 0)], 0.0
    ).astype(np.float32)

    # ---- launch B: layer-1 aggregate + epilogue -> z'
    rB = _get_runner(ncB, "B")
    argsB = rB.stage(_prep_inputs_B(h_pad, W2, b1, dinv_d, pp))
    resB = rB.run(argsB)
    zp = np.stack([r["zp"] for r in resB])  # [8, P, NBLK] f32
    z_pad = np.zeros(N_CORES * capc, np.float32)
    z_pad[pp["dest_id"][dvalid]] = zp[dvalid]

    # ---- launch C: layer-2 aggregate -> out
    rC = _get_runner(ncC, "C")
    argsC = rC.stage(_prep_inputs_C(z_pad, b2, dinv_d, pp))
    resC = rC.run(argsC)
    outv = np.stack([r["outv"] for r in resC])  # [8, P, NBLK] f32
    out = np.zeros(n_nodes, np.float32)
    out[pp["dest_id"][dvalid]] = outv[dvalid]
    _LAST.update(
        pp=pp, argsA=argsA, argsB=argsB, argsC=argsC,
        runners=(rA, rB, rC), programs=(ncA, ncB, ncC),
    )
    return out


# ---------------------------------------------------------------------------
# mini self-test on a small synthetic graph (same code path end-to-end)
# ---------------------------------------------------------------------------

if __name__ == "__main__":
    rng = np.random.default_rng(0)
    n, e = 3000, 40000
    x = rng.standard_normal((n, IN_DIM)).astype(np.float32)
    ei = rng.integers(0, n, (2, e)).astype(np.int64)
    W1 = (rng.standard_normal((IN_DIM, HID)) / np.sqrt(IN_DIM)).astype(
        np.float32
    )
    b1 = rng.standard_normal(HID).astype(np.float32) * 0.1
    W2 = (rng.standard_normal((HID, 1)) / np.sqrt(HID)).astype(np.float32)
    b2 = rng.standard_normal(1).astype(np.float32) * 0.1

    got = kernel(x, W1, b1, W2, b2, ei)

    # numpy reference
    dst = np.concatenate([ei[0], np.arange(n)])
    srcn = np.concatenate([ei[1], np.arange(n)])
    deg = np.bincount(dst, minlength=n).astype(np.float64)
    dinv = 1.0 / np.sqrt(deg)
    h = x.astype(np.float64) @ W1.astype(np.float64)
    msg = h[srcn] * (dinv[dst] * dinv[srcn])[:, None]
    agg = np.zeros((n, HID))
    np.add.at(agg, dst, msg)
    h1 = np.maximum(agg + b1, 0)
    z = h1 @ W2.astype(np.float64)
    msg2 = z[srcn, 0] * dinv[dst] * dinv[srcn]
    agg2 = np.zeros(n)
    np.add.at(agg2, dst, msg2)
    ref = (agg2 + b2[0]).astype(np.float32)

    rel = np.linalg.norm(got - ref) / np.linalg.norm(ref)
    print(f"mini self-test rel err: {rel:.6f}")
    assert rel < 2e-2, "mini self-test failed"
    print("mini self-test PASSED")
